# revision 6
# baseline (speedup 1.0000x reference)
"""GAT model (3 GATConv layers + mean-pool readout + MLP head) on 8 TRN2 cores.

Self-contained: host-side numpy builds the layout (degree-sorted node blocks,
slot-major edge padding), the Bass/Tile program runs projections, per-edge
indirect gathers, segment softmax, aggregation, readout and MLP on device.

Layout (see inline comments):
- Nodes sorted by in-degree desc, padded to NPAD rows = 128*KBLK*8; table row
  order is core-major so each core owns a contiguous 1/8 of rows.
- Block k (per core) has uniform slot count D[k] (max degree at that position
  across cores); slot 0 = self-loop, pads masked via -1e9 in the edge-attr
  attention term.
- Per layer: table_L[row] = [h | s | d] (projection with appended attention
  columns); per block: one indirect-DMA gather per slot (128 rows/instr).
"""
import os

import numpy as np

import concourse.bass as bass
import concourse.bacc as bacc
import concourse.mybir as mybir
import concourse.tile as tile
from concourse.bass_utils import run_bass_kernel_spmd

F32 = mybir.dt.float32
I32 = mybir.dt.int32

N = 40000
E = 640000
G = 128
HID = 64
HEADS = 4
NCORES = 8
NPAD = 40960
KBLK = NPAD // (128 * NCORES)  # 40
CORE_ROWS = KBLK * 128  # 5120
W12 = 264  # [h(256) | s(4) | d(4)]
W3 = 72    # [h(64) | s(1) | d(1) | pad(6)]
HC12 = 256
HC3 = 64


# ---------------------------------------------------------------- host layout

def build_layout(edge_index, batch):
    src0 = np.asarray(edge_index[0]).astype(np.int64)
    dst0 = np.asarray(edge_index[1]).astype(np.int64)
    batch = np.asarray(batch).astype(np.int64)
    ne = src0.size
    deg_in = np.bincount(dst0, minlength=N) + 1  # incl self loop

    order = np.argsort(-deg_in, kind="stable")
    g_of_rank = np.arange(NPAD) // 128
    row_of_rank = (
        (g_of_rank % NCORES) * CORE_ROWS
        + (g_of_rank // NCORES) * 128
        + np.arange(NPAD) % 128
    )
    node_of_row = np.full(NPAD, -1, dtype=np.int64)
    node_of_row[row_of_rank[:N]] = order
    row_of_node = np.empty(N, dtype=np.int64)
    row_of_node[order] = row_of_rank[:N]

    D = np.ones(KBLK, dtype=np.int64)
    for k in range(KBLK):
        fr = 8 * k * 128
        if fr < N:
            D[k] = max(1, deg_in[order[fr]])
    off = np.zeros(KBLK + 1, dtype=np.int64)
    off[1:] = np.cumsum(D)
    Dsum = int(off[-1])

    # edge slot assignment (slot 0 = self)
    src_rows = row_of_node[src0]
    dst_rows = row_of_node[dst0]
    eorder = np.argsort(dst_rows, kind="stable")
    ds = dst_rows[eorder]
    uq, st, ct = np.unique(ds, return_index=True, return_counts=True)
    slot = np.arange(ne) - np.repeat(st, ct) + 1

    core_of_row = np.arange(NPAD) // CORE_ROWS
    k_of_row = (np.arange(NPAD) % CORE_ROWS) // 128
    p_of_row = np.arange(NPAD) % 128
    ecore, ek, ep = core_of_row[ds], k_of_row[ds], p_of_row[ds]
    es = src_rows[eorder]

    # gather idx: [NCORES, 128, Dsum]; slot 0 = own row; pads -> row 0
    idx = np.zeros((NCORES, 128, Dsum), dtype=np.int32)
    own = np.arange(NPAD).reshape(NCORES, KBLK, 128)
    for k in range(KBLK):
        idx[:, :, off[k]] = own[:, k, :]
    gcol = off[ek] + slot
    idx[ecore, ep, gcol] = es.astype(np.int32)

    # mask [NCORES, 128, Dsum]: -1e9 where slot >= degree
    mask = np.zeros((NCORES, 128, Dsum), dtype=np.float32)
    nodes_of = node_of_row.reshape(NCORES, KBLK, 128)
    degs = np.where(nodes_of >= 0, deg_in[np.clip(nodes_of, 0, N - 1)], 0)
    for k in range(KBLK):
        jj = np.arange(D[k])[None, None, :]
        mask[:, :, off[k]: off[k + 1]] = np.where(jj < degs[:, k, :, None], 0.0, -1e9)

    # ea packed [NCORES, 128, Dsum*6], slot-major (j, f); slot0 zeros
    ea_packed = np.zeros((NCORES, 128, Dsum * 6), dtype=np.float32)

    cnt = deg_in - 1
    cntv = np.where(nodes_of >= 0, cnt[np.clip(nodes_of, 0, N - 1)], 0)
    cnt_inv = (1.0 / np.maximum(cntv, 1)).astype(np.float32).transpose(0, 2, 1)  # [NC,128,KBLK]

    gids = np.where(nodes_of >= 0, batch[np.clip(nodes_of, 0, N - 1)], -1)
    onehot = np.zeros((NCORES, KBLK, 128, G), dtype=np.float32)
    ii = np.indices((NCORES, KBLK, 128))
    val = gids >= 0
    onehot[ii[0][val], ii[1][val], ii[2][val], gids[val]] = 1.0

    gcnt = np.bincount(batch, minlength=G).astype(np.float32)
    gcnt_inv = (1.0 / np.maximum(gcnt, 1.0)).astype(np.float32)

    return dict(order=order, row_of_node=row_of_node, node_of_row=node_of_row,
                D=D, off=off, Dsum=Dsum, idx=idx, mask=mask, cnt_inv=cnt_inv,
                onehot=onehot, gcnt_inv=gcnt_inv, eorder=eorder, ecore=ecore,
                ep=ep, gcol=gcol, deg_in=deg_in)


def fill_ea(L, edge_attr):
    ea_sorted = np.asarray(edge_attr, np.float32)[L["eorder"]]
    ea = np.zeros((NCORES, 128, L["Dsum"] * 6), dtype=np.float32)
    for f in range(6):
        ea[L["ecore"], L["ep"], L["gcol"] * 6 + f] = ea_sorted[:, f]
    return ea


def derived_weights(ip):
    def mk(W, a_s, a_d, heads, c, wout):
        W = np.asarray(W, np.float32)
        A_s = (W.reshape(-1, heads, c) * np.asarray(a_s)[None]).sum(-1)
        A_d = (W.reshape(-1, heads, c) * np.asarray(a_d)[None]).sum(-1)
        rhs = np.zeros((W.shape[0], wout), dtype=np.float32)
        rhs[:, : heads * c] = W
        rhs[:, heads * c: heads * c + heads] = A_s
        rhs[:, heads * c + heads: heads * c + 2 * heads] = A_d
        return rhs
    rhs1 = mk(ip["W1"], ip["a_s1"], ip["a_d1"], 4, 64, W12)
    rhs2 = mk(ip["W2"], ip["a_s2"], ip["a_d2"], 4, 64, W12)
    rhs3 = mk(ip["W3"], ip["a_s3"], ip["a_d3"], 1, 64, W3)
    Ve1 = (np.asarray(ip["We1"], np.float32).reshape(6, 4, 64)
           * np.asarray(ip["a_e1"])[None]).sum(-1)  # [6,4]
    Ve2 = (np.asarray(ip["We2"], np.float32).reshape(6, 4, 64)
           * np.asarray(ip["a_e2"])[None]).sum(-1)
    Ve3 = (np.asarray(ip["We3"], np.float32).reshape(6, 1, 64)
           * np.asarray(ip["a_e3"])[None]).sum(-1)  # [6,1]
    # ve_rows [6, 9]: per input feature f, the 9 (layer,head) coefficients
    ve9 = np.concatenate([Ve1, Ve2, Ve3], axis=1).astype(np.float32)  # [6, 9]
    return rhs1, rhs2, rhs3, ve9


# ---------------------------------------------------------------- bass build

_CACHE = {}
last_exec_time_ns = None


def build_program(L):
    D, off, Dsum = L["D"], L["off"], L["Dsum"]
    nc = bacc.Bacc("TRN2", target_bir_lowering=False, debug=False,
                   num_devices=NCORES)

    # ---- inputs
    xt_in = nc.dram_tensor("xt", [7, NPAD], F32, kind="ExternalInput")
    rhs1_in = nc.dram_tensor("rhs1", [7, W12], F32, kind="ExternalInput")
    rhs2_in = nc.dram_tensor("rhs2", [256, W12], F32, kind="ExternalInput")
    rhs3_in = nc.dram_tensor("rhs3", [256, W3], F32, kind="ExternalInput")
    ve9_in = nc.dram_tensor("ve9", [128, 54], F32, kind="ExternalInput")
    idx_in = nc.dram_tensor("idx", [128, Dsum], I32, kind="ExternalInput")
    mask_in = nc.dram_tensor("mask", [128, Dsum], F32, kind="ExternalInput")
    ea_in = nc.dram_tensor("ea", [128, Dsum * 6], F32, kind="ExternalInput")
    cinv_in = nc.dram_tensor("cinv", [128, KBLK], F32, kind="ExternalInput")
    oneh_in = nc.dram_tensor("oneh", [KBLK * 128, G], F32, kind="ExternalInput")
    ginv_in = nc.dram_tensor("ginv", [G, 1], F32, kind="ExternalInput")
    b1_in = nc.dram_tensor("b1t", [128, 256], F32, kind="ExternalInput")
    b2_in = nc.dram_tensor("b2t", [128, 256], F32, kind="ExternalInput")
    b3_in = nc.dram_tensor("b3t", [128, 64], F32, kind="ExternalInput")
    ident_in = nc.dram_tensor("ident", [128, 128], F32, kind="ExternalInput")
    p1w_in = nc.dram_tensor("p1w", [64, 64], F32, kind="ExternalInput")
    p2w_in = nc.dram_tensor("p2w", [64, 32], F32, kind="ExternalInput")
    p3w_in = nc.dram_tensor("p3w", [32, 1], F32, kind="ExternalInput")
    p1b_in = nc.dram_tensor("p1bt", [G, 64], F32, kind="ExternalInput")
    p2b_in = nc.dram_tensor("p2bt", [G, 32], F32, kind="ExternalInput")
    p3b_in = nc.dram_tensor("p3bt", [G, 1], F32, kind="ExternalInput")

    ro_out = nc.dram_tensor("readout", [G, 64], F32, kind="ExternalOutput")
    og_out = nc.dram_tensor("out_g", [G, 1], F32, kind="ExternalOutput")

    AX = mybir.AxisListType.X
    AF = mybir.ActivationFunctionType

    with tile.TileContext(nc) as tc:
        with (
            tc.tile_pool(name="dram", bufs=1, space="DRAM") as dpool,
            tc.tile_pool(name="persist", bufs=1) as pp,
            tc.tile_pool(name="work", bufs=3) as wp,
            tc.tile_pool(name="projp", bufs=2) as jp,
            tc.tile_pool(name="gath", bufs=2) as gp,
            tc.tile_pool(name="psum", bufs=2, space="PSUM") as psp,
            tc.tile_pool(name="psum1", bufs=1, space="PSUM") as ps1,
        ):
            table1 = dpool.tile([NPAD, W12], F32)
            table2 = dpool.tile([NPAD, W12], F32)
            table3 = dpool.tile([NPAD, W3], F32)
            myslab = dpool.tile([256, CORE_ROWS], F32)
            yt = dpool.tile([NCORES, 256, CORE_ROWS], F32)
            ar_in = dpool.tile([G, 64], F32)
            ar_out = dpool.tile([G, 64], F32)

            # ---- persistent SBUF
            idx_sb = pp.tile([128, Dsum], I32)
            nc.sync.dma_start(out=idx_sb[:], in_=idx_in[:])
            mask_sb = pp.tile([128, Dsum], F32)
            nc.sync.dma_start(out=mask_sb[:], in_=mask_in[:])
            cinv_sb = pp.tile([128, KBLK], F32)
            nc.sync.dma_start(out=cinv_sb[:], in_=cinv_in[:])
            ve_sb = pp.tile([128, 54], F32)
            nc.sync.dma_start(out=ve_sb[:], in_=ve9_in[:])
            t9 = pp.tile([128, 9 * Dsum], F32)
            b1_sb = pp.tile([128, 256], F32)
            nc.sync.dma_start(out=b1_sb[:], in_=b1_in[:])
            b2_sb = pp.tile([128, 256], F32)
            nc.sync.dma_start(out=b2_sb[:], in_=b2_in[:])
            b3_sb = pp.tile([128, 64], F32)
            nc.sync.dma_start(out=b3_sb[:], in_=b3_in[:])
            ident_sb = pp.tile([128, 128], F32)
            nc.sync.dma_start(out=ident_sb[:], in_=ident_in[:])
            rhs1_sb = pp.tile([7, W12], F32)
            nc.sync.dma_start(out=rhs1_sb[:], in_=rhs1_in[:])
            rhs2a_sb = pp.tile([128, W12], F32)
            nc.sync.dma_start(out=rhs2a_sb[:], in_=rhs2_in[0:128, :])
            rhs2b_sb = pp.tile([128, W12], F32)
            nc.sync.dma_start(out=rhs2b_sb[:], in_=rhs2_in[128:256, :])
            rhs3a_sb = pp.tile([128, W3], F32)
            nc.sync.dma_start(out=rhs3a_sb[:], in_=rhs3_in[0:128, :])
            rhs3b_sb = pp.tile([128, W3], F32)
            nc.sync.dma_start(out=rhs3b_sb[:], in_=rhs3_in[128:256, :])
            ginv_sb = pp.tile([G, 1], F32)
            nc.sync.dma_start(out=ginv_sb[:], in_=ginv_in[:])
            p1w_sb = pp.tile([64, 64], F32)
            nc.sync.dma_start(out=p1w_sb[:], in_=p1w_in[:])
            p2w_sb = pp.tile([64, 32], F32)
            nc.sync.dma_start(out=p2w_sb[:], in_=p2w_in[:])
            p3w_sb = pp.tile([32, 1], F32)
            nc.sync.dma_start(out=p3w_sb[:], in_=p3w_in[:])
            p1b_sb = pp.tile([G, 64], F32)
            nc.sync.dma_start(out=p1b_sb[:], in_=p1b_in[:])
            p2b_sb = pp.tile([G, 32], F32)
            nc.sync.dma_start(out=p2b_sb[:], in_=p2b_in[:])
            p3b_sb = pp.tile([G, 1], F32)
            nc.sync.dma_start(out=p3b_sb[:], in_=p3b_in[:])

            # ---- t9 pass: t9[p, l9, j] per block; l9 = (L1 h0..3, L2 h0..3, L3)
            for k in range(KBLK):
                dk = int(D[k])
                ea_k = wp.tile([128, dk * 6], F32, tag="ea")
                nc.sync.dma_start(out=ea_k[:], in_=ea_in[:, off[k] * 6: off[k + 1] * 6])
                tk = t9[:].rearrange("p (n j) -> p n j", n=9)[:, :, off[k]: off[k + 1]]
                tmp = wp.tile([128, 9 * dk], F32, tag="ttmp")
                tmpv = tmp[:].rearrange("p (n j) -> p n j", n=9)
                eav = ea_k[:].rearrange("p (j f) -> p f j", f=6)
                for f in range(6):
                    eab = eav[:, f, :].to_broadcast([128, dk, 9]).rearrange("p j n -> p n j")
                    veb = ve_sb[:, f * 9: (f + 1) * 9].to_broadcast([128, 9, dk])
                    if f == 0:
                        nc.vector.tensor_mul(tk, eab, veb)
                    else:
                        nc.vector.tensor_mul(tmpv, eab, veb)
                        nc.vector.tensor_add(tk, tk, tmpv)
                # self-loop slot 0 = (sum over slots) * cnt_inv
                tsum = wp.tile([128, 9], F32, tag="tsum")
                nc.vector.reduce_sum(tsum[:], tk, axis=AX)
                nc.vector.tensor_scalar(
                    out=tk[:, :, 0:1].rearrange("p n j -> p (n j)"),
                    in0=tsum[:], scalar1=cinv_sb[:, k: k + 1], scalar2=None,
                    op0=mybir.AluOpType.mult)
                # bake pad mask (broadcast over the 9 lanes)
                mb = mask_sb[:, off[k]: off[k + 1]].to_broadcast([128, dk, 9]).rearrange("p j n -> p n j")
                nc.vector.tensor_add(tk, tk, mb)

            # ---- per layer
            def projection(layer):
                wl = W12 if layer < 3 else W3
                tbl = (table1, table2, table3)[layer - 1]
                if layer == 1:
                    CH = min(2048, NPAD)
                    for ci in range(NPAD // CH):
                        xt_c = jp.tile([7, CH], F32, tag="xtc")
                        nc.sync.dma_start(out=xt_c[:], in_=xt_in[:, ci * CH: (ci + 1) * CH])
                        for m in range(CH // 128):
                            ps = psp.tile([128, wl], F32, tag="proj", space="PSUM")
                            nc.tensor.matmul(ps[:], lhsT=xt_c[:, m * 128: (m + 1) * 128],
                                             rhs=rhs1_sb[:], start=True, stop=True)
                            ot = jp.tile([128, wl], F32, tag="projo")
                            nc.vector.tensor_copy(ot[:], ps[:])
                            r0 = ci * CH + m * 128
                            nc.sync.dma_start(out=tbl[r0: r0 + 128, :], in_=ot[:])
                else:
                    ra, rb = (rhs2a_sb, rhs2b_sb) if layer == 2 else (rhs3a_sb, rhs3b_sb)
                    CH = min(1280, CORE_ROWS)
                    for c2 in range(NCORES):
                        for ci in range(CORE_ROWS // CH):
                            la = jp.tile([128, CH], F32, tag="lha")
                            lb = jp.tile([128, CH], F32, tag="lhb")
                            nc.sync.dma_start(out=la[:], in_=yt[c2, 0:128, ci * CH: (ci + 1) * CH])
                            nc.sync.dma_start(out=lb[:], in_=yt[c2, 128:256, ci * CH: (ci + 1) * CH])
                            for m in range(CH // 128):
                                ps = psp.tile([128, wl], F32, tag="proj", space="PSUM")
                                nc.tensor.matmul(ps[:], lhsT=la[:, m * 128: (m + 1) * 128],
                                                 rhs=ra[:], start=True, stop=False)
                                nc.tensor.matmul(ps[:], lhsT=lb[:, m * 128: (m + 1) * 128],
                                                 rhs=rb[:], start=False, stop=True)
                                ot = jp.tile([128, wl], F32, tag="projo")
                                nc.vector.tensor_copy(ot[:], ps[:])
                                r0 = c2 * CORE_ROWS + ci * CH + m * 128
                                nc.sync.dma_start(out=tbl[r0: r0 + 128, :], in_=ot[:])

            def agg_layer(layer):
                wl, hl, hc = (W12, 4, 256) if layer < 3 else (W3, 1, 64)
                tbl = (table1, table2, table3)[layer - 1]
                l9o = 0 if layer == 1 else (4 if layer == 2 else 8)
                for k in range(KBLK):
                    dk = int(D[k])
                    hs = gp.tile([128, dk * wl], F32, tag="hs")
                    for j in range(dk):
                        nc.gpsimd.indirect_dma_start(
                            out=hs[:, j * wl: (j + 1) * wl], out_offset=None,
                            in_=tbl[:],
                            in_offset=bass.IndirectOffsetOnAxis(
                                ap=idx_sb[:, off[k] + j: off[k] + j + 1], axis=0))
                    hsv = hs[:].rearrange("p (j w) -> p j w", w=wl)
                    # alpha = t + s[src] + d[own]
                    a = wp.tile([128, hl * dk], F32, tag="alpha")
                    av = a[:].rearrange("p (h j) -> p h j", h=hl)
                    tk = t9[:].rearrange("p (n j) -> p n j", n=9)[:, l9o: l9o + hl,
                                                                 off[k]: off[k + 1]]
                    sv = hsv[:, :, hc: hc + hl].rearrange("p j h -> p h j")
                    nc.vector.tensor_add(av, tk, sv)
                    for h in range(hl):
                        nc.vector.tensor_scalar_add(
                            av[:, h, :], av[:, h, :],
                            hsv[:, 0, hc + hl + h: hc + hl + h + 1])
                    lr = wp.tile([128, hl * dk], F32, tag="lr")
                    nc.vector.tensor_scalar_mul(lr[:], a[:], 0.2)
                    nc.vector.tensor_max(a[:], a[:], lr[:])
                    mx = wp.tile([128, hl], F32, tag="mx")
                    nc.vector.reduce_max(mx[:], av, axis=AX)
                    negm = wp.tile([128, hl], F32, tag="negm")
                    nc.vector.tensor_scalar_mul(negm[:], mx[:], -1.0)
                    e = wp.tile([128, hl * dk], F32, tag="e")
                    ev = e[:].rearrange("p (h j) -> p h j", h=hl)
                    for h in range(hl):
                        nc.scalar.activation(e[:, h * dk: (h + 1) * dk],
                                             a[:, h * dk: (h + 1) * dk],
                                             AF.Exp, bias=negm[:, h: h + 1])
                    ssum = wp.tile([128, hl], F32, tag="ssum")
                    nc.vector.reduce_sum(ssum[:], ev, axis=AX)
                    nc.vector.tensor_scalar_add(ssum[:], ssum[:], 1e-16)
                    rinv = wp.tile([128, hl], F32, tag="rinv")
                    nc.vector.reciprocal(rinv[:], ssum[:])
                    # weighted sum over slots
                    prod_in = hsv[:, :, 0:hc]
                    eb = ev.rearrange("p h j -> p j h").to_broadcast([128, dk, hl, 64]) \
                        if hl > 1 else ev.rearrange("p h j -> p j h").to_broadcast([128, dk, 1, 64])
                    pv = prod_in.rearrange("p j (h c) -> p j h c", h=hl)
                    nc.vector.tensor_mul(pv, pv, eb)
                    n = dk
                    while n > 1:
                        if n % 2 == 1:
                            nc.vector.tensor_add(hsv[:, 0, 0:hc], hsv[:, 0, 0:hc],
                                                 hsv[:, n - 1, 0:hc])
                            n -= 1
                        half = n // 2
                        nc.vector.tensor_add(hsv[:, 0:half, 0:hc],
                                             hsv[:, 0:half, 0:hc],
                                             hsv[:, half: n, 0:hc])
                        n = half
                    y = wp.tile([128, hc], F32, tag="y")
                    rb = rinv[:].to_broadcast([128, hl, 64])
                    nc.vector.tensor_mul(y[:].rearrange("p (h c) -> p h c", c=64),
                                         hsv[:, 0, 0:hc].rearrange("p (h c) -> p h c", c=64),
                                         rb)
                    # bias + elu
                    bt = (b1_sb, b2_sb, b3_sb)[layer - 1]
                    nc.vector.tensor_add(y[:], y[:], bt[:, 0:hc])
                    u = wp.tile([128, hc], F32, tag="u")
                    nc.vector.tensor_scalar_min(u[:], y[:], 0.0)
                    eu = wp.tile([128, hc], F32, tag="eu")
                    nc.scalar.activation(eu[:], u[:], AF.Exp)
                    nc.vector.tensor_scalar_max(y[:], y[:], 0.0)
                    nc.vector.tensor_add(y[:], y[:], eu[:])
                    nc.vector.tensor_scalar_add(y[:], y[:], -1.0)
                    if layer < 3:
                        # transpose -> myslab columns [256, 128]
                        for half in range(2):
                            tp = psp.tile([128, 128], F32, tag="tp", space="PSUM")
                            nc.tensor.transpose(tp[:], y[:, half * 128: (half + 1) * 128],
                                                ident_sb[:])
                            to = wp.tile([128, 128], F32, tag="tpo")
                            nc.vector.tensor_copy(to[:], tp[:])
                            nc.sync.dma_start(
                                out=myslab[half * 128: (half + 1) * 128,
                                           k * 128: (k + 1) * 128],
                                in_=to[:])
                    else:
                        oh = wp.tile([128, G], F32, tag="oh")
                        nc.sync.dma_start(out=oh[:], in_=oneh_in[k * 128: (k + 1) * 128, :])
                        nc.tensor.matmul(racc_ps[:], lhsT=oh[:], rhs=y[:],
                                         start=(k == 0), stop=(k == KBLK - 1))

            # ===== layer 1
            projection(1)
            agg_layer(1)
            nc.gpsimd.collective_compute(
                "AllGather", mybir.AluOpType.bypass,
                replica_groups=[list(range(NCORES))],
                ins=[myslab.opt()], outs=[yt.opt()])
            # ===== layer 2
            projection(2)
            agg_layer(2)
            nc.gpsimd.collective_compute(
                "AllGather", mybir.AluOpType.bypass,
                replica_groups=[list(range(NCORES))],
                ins=[myslab.opt()], outs=[yt.opt()])
            # ===== layer 3 + readout accumulation
            projection(3)
            racc_ps = ps1.tile([G, 64], F32, tag="racc", space="PSUM")
            agg_layer(3)
            racc = wp.tile([G, 64], F32, tag="racc_sb")
            nc.vector.tensor_copy(racc[:], racc_ps[:])
            nc.sync.dma_start(out=ar_in[:], in_=racc[:])
            nc.gpsimd.collective_compute(
                "AllReduce", mybir.AluOpType.add,
                replica_groups=[list(range(NCORES))],
                ins=[ar_in.opt()], outs=[ar_out.opt()])
            ro = wp.tile([G, 64], F32, tag="ro")
            nc.sync.dma_start(out=ro[:], in_=ar_out[:])
            nc.vector.tensor_scalar(out=ro[:], in0=ro[:], scalar1=ginv_sb[:, 0:1],
                                    scalar2=None, op0=mybir.AluOpType.mult)
            nc.sync.dma_start(out=ro_out[:], in_=ro[:])
            # ---- MLP head (replicated on every core)
            def ptranspose(src_ap, cols, parts):
                tp = psp.tile([128, 128], F32, tag="tp", space="PSUM")
                nc.tensor.transpose(tp[:cols, :parts], src_ap, ident_sb[:parts, :parts])
                to = wp.tile([128, 128], F32, tag="tpo")
                nc.vector.tensor_copy(to[:cols, :parts], tp[:cols, :parts])
                return to

            rt = ptranspose(ro[:], 64, G)
            z1ps = psp.tile([G, 64], F32, tag="mlp", space="PSUM")
            nc.tensor.matmul(z1ps[:], lhsT=rt[:64, :G], rhs=p1w_sb[:], start=True, stop=True)
            z1 = wp.tile([G, 64], F32, tag="z1")
            nc.vector.tensor_add(z1[:], z1ps[:], p1b_sb[:])
            nc.scalar.activation(z1[:], z1[:], AF.Relu)
            z1t = ptranspose(z1[:], 64, G)
            z2ps = psp.tile([G, 32], F32, tag="mlp", space="PSUM")
            nc.tensor.matmul(z2ps[:], lhsT=z1t[:64, :G], rhs=p2w_sb[:], start=True, stop=True)
            z2 = wp.tile([G, 32], F32, tag="z2")
            nc.vector.tensor_add(z2[:], z2ps[:], p2b_sb[:])
            nc.scalar.activation(z2[:], z2[:], AF.Relu)
            z2t = ptranspose(z2[:], 32, G)
            z3ps = psp.tile([G, 1], F32, tag="mlp", space="PSUM")
            nc.tensor.matmul(z3ps[:], lhsT=z2t[:32, :G], rhs=p3w_sb[:], start=True, stop=True)
            z3 = wp.tile([G, 1], F32, tag="z3")
            nc.vector.tensor_add(z3[:], z3ps[:], p3b_sb[:])
            nc.sync.dma_start(out=og_out[:], in_=z3[:])

    nc.compile()
    return nc


# ---------------------------------------------------------------- entry point

def kernel(**inputs):
    ip = {k: np.asarray(v) for k, v in inputs.items()}
    L = build_layout(ip["edge_index"], ip["batch"])
    rhs1, rhs2, rhs3, ve9 = derived_weights(ip)
    ea = fill_ea(L, ip["edge_attr"])

    x_rows = np.zeros((NPAD, 7), np.float32)
    valid = L["node_of_row"] >= 0
    x_rows[valid] = np.asarray(ip["x"], np.float32)[L["node_of_row"][valid]]
    xt = np.ascontiguousarray(x_rows.T)

    key = "prog"
    if key not in _CACHE:
        _CACHE[key] = build_program(L)
    nc = _CACHE[key]

    rep = lambda v, w: np.broadcast_to(np.asarray(v, np.float32)[None, :], (128, w)).copy()
    repg = lambda v, w: np.broadcast_to(np.asarray(v, np.float32)[None, :], (G, w)).copy()
    shared = {
        "xt": xt, "rhs1": rhs1, "rhs2": rhs2, "rhs3": rhs3,
        "ve9": np.broadcast_to(ve9.reshape(1, 54), (128, 54)).copy(),
        "ginv": L["gcnt_inv"].reshape(G, 1),
        "b1t": rep(ip["b1"], 256), "b2t": rep(ip["b2"], 256),
        "b3t": rep(ip["b3"], 64),
        "ident": np.eye(128, dtype=np.float32),
        "p1w": np.asarray(ip["p1w"], np.float32),
        "p2w": np.asarray(ip["p2w"], np.float32),
        "p3w": np.asarray(ip["p3w"], np.float32),
        "p1bt": repg(ip["p1b"], 64), "p2bt": repg(ip["p2b"], 32),
        "p3bt": repg(ip["p3b"], 1),
    }
    in_maps = []
    for c in range(NCORES):
        m = dict(shared)
        m["idx"] = L["idx"][c]
        m["mask"] = L["mask"][c]
        m["ea"] = ea[c]
        m["cinv"] = L["cnt_inv"][c]
        m["oneh"] = L["onehot"][c].reshape(KBLK * 128, G)
        in_maps.append(m)

    trace = bool(os.environ.get("GAT_TRACE"))
    res = run_bass_kernel_spmd(nc, in_maps, list(range(NCORES)), trace=trace)
    global last_exec_time_ns
    last_exec_time_ns = res.exec_time_ns
    out = res.results[0]["out_g"][:, 0]
    readout = res.results[0]["readout"]
    return out, readout


if __name__ == "__main__":
    dat = np.load("/root/problem/inputs.npz")
    inputs = {k: dat[k] for k in dat.files}
    exp = np.load("/root/problem/expected.npz")
    out, readout = kernel(**inputs)
    ro = np.abs(readout - exp["readout"]).max() / np.abs(exp["readout"]).max()
    oo = np.abs(out - exp["out"]).max() / np.abs(exp["out"]).max()
    print("readout rel err:", ro)
    print("out rel err:", oo)


# revision 8
# speedup vs baseline: 1.3131x; 1.3131x over previous
"""GAT model (3 GATConv layers + mean-pool readout + MLP head) on 8 TRN2 cores.

Self-contained: host-side numpy builds the layout (degree-sorted node blocks,
slot-major edge padding), the Bass/Tile program runs projections, per-edge
indirect gathers, segment softmax, aggregation, readout and MLP on device.

Layout (see inline comments):
- Nodes sorted by in-degree desc, padded to NPAD rows = 128*KBLK*8; table row
  order is core-major so each core owns a contiguous 1/8 of rows.
- Block k (per core) has uniform slot count D[k] (max degree at that position
  across cores); slot 0 = self-loop, pads masked via -1e9 in the edge-attr
  attention term.
- Per layer: table_L[row] = [h | s | d] (projection with appended attention
  columns); per block: one indirect-DMA gather per slot (128 rows/instr).
"""
import os

import numpy as np

import concourse.bass as bass
import concourse.bacc as bacc
import concourse.mybir as mybir
import concourse.tile as tile
from concourse.bass_utils import run_bass_kernel_spmd

F32 = mybir.dt.float32
I32 = mybir.dt.int32

N = 40000
E = 640000
G = 128
HID = 64
HEADS = 4
NCORES = 8
NPAD = 40960
KBLK = NPAD // (128 * NCORES)  # 40
CORE_ROWS = KBLK * 128  # 5120
AGCH = 4              # AllGather chunks per layer table
KCH = KBLK // AGCH    # k-positions per chunk
W12 = 264  # [h(256) | s(4) | d(4)]
W3 = 72    # [h(64) | s(1) | d(1) | pad(6)]
HC12 = 256
HC3 = 64


# ---------------------------------------------------------------- host layout

def build_layout(edge_index, batch):
    src0 = np.asarray(edge_index[0]).astype(np.int64)
    dst0 = np.asarray(edge_index[1]).astype(np.int64)
    batch = np.asarray(batch).astype(np.int64)
    ne = src0.size
    deg_in = np.bincount(dst0, minlength=N) + 1  # incl self loop

    order = np.argsort(-deg_in, kind="stable")
    g_of_rank = np.arange(NPAD) // 128
    c_of_rank = g_of_rank % NCORES
    k_of_rank = g_of_rank // NCORES
    # position-major row order: chunks of KCH k-positions, core-major within
    row_of_rank = (
        (k_of_rank // KCH) * (NCORES * KCH * 128)
        + c_of_rank * (KCH * 128)
        + (k_of_rank % KCH) * 128
        + np.arange(NPAD) % 128
    )
    node_of_row = np.full(NPAD, -1, dtype=np.int64)
    node_of_row[row_of_rank[:N]] = order
    row_of_node = np.empty(N, dtype=np.int64)
    row_of_node[order] = row_of_rank[:N]

    D = np.ones(KBLK, dtype=np.int64)
    for k in range(KBLK):
        fr = 8 * k * 128
        if fr < N:
            D[k] = max(1, deg_in[order[fr]])
    off = np.zeros(KBLK + 1, dtype=np.int64)
    off[1:] = np.cumsum(D)
    Dsum = int(off[-1])

    # edge slot assignment (slot 0 = self)
    src_rows = row_of_node[src0]
    dst_rows = row_of_node[dst0]
    eorder = np.argsort(dst_rows, kind="stable")
    ds = dst_rows[eorder]
    uq, st, ct = np.unique(ds, return_index=True, return_counts=True)
    slot = np.arange(ne) - np.repeat(st, ct) + 1

    rr = np.arange(NPAD)
    core_of_row = (rr % (NCORES * KCH * 128)) // (KCH * 128)
    k_of_row = (rr // (NCORES * KCH * 128)) * KCH + (rr % (KCH * 128)) // 128
    p_of_row = rr % 128
    ecore, ek, ep = core_of_row[ds], k_of_row[ds], p_of_row[ds]
    es = src_rows[eorder]

    # gather idx: [NCORES, 128, Dsum]; slot 0 = own row; pads -> row 0
    idx = np.zeros((NCORES, 128, Dsum), dtype=np.int32)
    own = np.arange(NPAD).reshape(AGCH, NCORES, KCH, 128).transpose(
        1, 0, 2, 3).reshape(NCORES, KBLK, 128)
    for k in range(KBLK):
        idx[:, :, off[k]] = own[:, k, :]
    gcol = off[ek] + slot
    idx[ecore, ep, gcol] = es.astype(np.int32)

    # mask [NCORES, 128, Dsum]: -1e9 where slot >= degree
    mask = np.zeros((NCORES, 128, Dsum), dtype=np.float32)
    nodes_of = node_of_row.reshape(AGCH, NCORES, KCH, 128).transpose(
        1, 0, 2, 3).reshape(NCORES, KBLK, 128)
    degs = np.where(nodes_of >= 0, deg_in[np.clip(nodes_of, 0, N - 1)], 0)
    for k in range(KBLK):
        jj = np.arange(D[k])[None, None, :]
        mask[:, :, off[k]: off[k + 1]] = np.where(jj < degs[:, k, :, None], 0.0, -1e9)

    # ea packed [NCORES, 128, Dsum*6], slot-major (j, f); slot0 zeros
    ea_packed = np.zeros((NCORES, 128, Dsum * 6), dtype=np.float32)

    cnt = deg_in - 1
    cntv = np.where(nodes_of >= 0, cnt[np.clip(nodes_of, 0, N - 1)], 0)
    cnt_inv = (1.0 / np.maximum(cntv, 1)).astype(np.float32).transpose(0, 2, 1)  # [NC,128,KBLK]

    gids = np.where(nodes_of >= 0, batch[np.clip(nodes_of, 0, N - 1)], -1)
    onehot = np.zeros((NCORES, KBLK, 128, G), dtype=np.float32)
    ii = np.indices((NCORES, KBLK, 128))
    val = gids >= 0
    onehot[ii[0][val], ii[1][val], ii[2][val], gids[val]] = 1.0

    gcnt = np.bincount(batch, minlength=G).astype(np.float32)
    gcnt_inv = (1.0 / np.maximum(gcnt, 1.0)).astype(np.float32)

    return dict(order=order, row_of_node=row_of_node, node_of_row=node_of_row,
                D=D, off=off, Dsum=Dsum, idx=idx, mask=mask, cnt_inv=cnt_inv,
                onehot=onehot, gcnt_inv=gcnt_inv, eorder=eorder, ecore=ecore,
                ep=ep, gcol=gcol, deg_in=deg_in)


def fill_ea(L, edge_attr):
    ea_sorted = np.asarray(edge_attr, np.float32)[L["eorder"]]
    ea = np.zeros((NCORES, 128, L["Dsum"] * 6), dtype=np.float32)
    for f in range(6):
        ea[L["ecore"], L["ep"], L["gcol"] * 6 + f] = ea_sorted[:, f]
    return ea


def derived_weights(ip):
    def mk(W, a_s, a_d, heads, c, wout):
        W = np.asarray(W, np.float32)
        A_s = (W.reshape(-1, heads, c) * np.asarray(a_s)[None]).sum(-1)
        A_d = (W.reshape(-1, heads, c) * np.asarray(a_d)[None]).sum(-1)
        rhs = np.zeros((W.shape[0], wout), dtype=np.float32)
        rhs[:, : heads * c] = W
        rhs[:, heads * c: heads * c + heads] = A_s
        rhs[:, heads * c + heads: heads * c + 2 * heads] = A_d
        return rhs
    rhs1 = mk(ip["W1"], ip["a_s1"], ip["a_d1"], 4, 64, W12)
    rhs2 = mk(ip["W2"], ip["a_s2"], ip["a_d2"], 4, 64, W12)
    rhs3 = mk(ip["W3"], ip["a_s3"], ip["a_d3"], 1, 64, W3)
    Ve1 = (np.asarray(ip["We1"], np.float32).reshape(6, 4, 64)
           * np.asarray(ip["a_e1"])[None]).sum(-1)  # [6,4]
    Ve2 = (np.asarray(ip["We2"], np.float32).reshape(6, 4, 64)
           * np.asarray(ip["a_e2"])[None]).sum(-1)
    Ve3 = (np.asarray(ip["We3"], np.float32).reshape(6, 1, 64)
           * np.asarray(ip["a_e3"])[None]).sum(-1)  # [6,1]
    # ve_rows [6, 9]: per input feature f, the 9 (layer,head) coefficients
    ve9 = np.concatenate([Ve1, Ve2, Ve3], axis=1).astype(np.float32)  # [6, 9]
    return rhs1, rhs2, rhs3, ve9


# ---------------------------------------------------------------- bass build

_CACHE = {}
last_exec_time_ns = None


def build_program(L):
    D, off, Dsum = L["D"], L["off"], L["Dsum"]
    nc = bacc.Bacc("TRN2", target_bir_lowering=False, debug=False,
                   num_devices=NCORES)

    # ---- inputs
    xt_in = nc.dram_tensor("xt", [7, NPAD], F32, kind="ExternalInput")
    rhs1_in = nc.dram_tensor("rhs1", [7, W12], F32, kind="ExternalInput")
    rhs2_in = nc.dram_tensor("rhs2", [256, W12], F32, kind="ExternalInput")
    rhs3_in = nc.dram_tensor("rhs3", [256, W3], F32, kind="ExternalInput")
    ve9_in = nc.dram_tensor("ve9", [128, 54], F32, kind="ExternalInput")
    idx_in = nc.dram_tensor("idx", [128, Dsum], I32, kind="ExternalInput")
    mask_in = nc.dram_tensor("mask", [128, Dsum], F32, kind="ExternalInput")
    ea_in = nc.dram_tensor("ea", [128, Dsum * 6], F32, kind="ExternalInput")
    cinv_in = nc.dram_tensor("cinv", [128, KBLK], F32, kind="ExternalInput")
    oneh_in = nc.dram_tensor("oneh", [KBLK * 128, G], F32, kind="ExternalInput")
    ginv_in = nc.dram_tensor("ginv", [G, 1], F32, kind="ExternalInput")
    b1_in = nc.dram_tensor("b1t", [128, 256], F32, kind="ExternalInput")
    b2_in = nc.dram_tensor("b2t", [128, 256], F32, kind="ExternalInput")
    b3_in = nc.dram_tensor("b3t", [128, 64], F32, kind="ExternalInput")
    ident_in = nc.dram_tensor("ident", [128, 128], F32, kind="ExternalInput")
    p1w_in = nc.dram_tensor("p1w", [64, 64], F32, kind="ExternalInput")
    p2w_in = nc.dram_tensor("p2w", [64, 32], F32, kind="ExternalInput")
    p3w_in = nc.dram_tensor("p3w", [32, 1], F32, kind="ExternalInput")
    p1b_in = nc.dram_tensor("p1bt", [G, 64], F32, kind="ExternalInput")
    p2b_in = nc.dram_tensor("p2bt", [G, 32], F32, kind="ExternalInput")
    p3b_in = nc.dram_tensor("p3bt", [G, 1], F32, kind="ExternalInput")

    ro_out = nc.dram_tensor("readout", [G, 64], F32, kind="ExternalOutput")
    og_out = nc.dram_tensor("out_g", [G, 1], F32, kind="ExternalOutput")

    AX = mybir.AxisListType.X
    AF = mybir.ActivationFunctionType

    with tile.TileContext(nc) as tc:
        with (
            tc.tile_pool(name="dram", bufs=1, space="DRAM") as dpool,
            tc.tile_pool(name="persist", bufs=1) as pp,
            tc.tile_pool(name="work", bufs=3) as wp,
            tc.tile_pool(name="projp", bufs=2) as jp,
            tc.tile_pool(name="gath", bufs=2) as gp,
            tc.tile_pool(name="psum", bufs=2, space="PSUM") as psp,
            tc.tile_pool(name="psum1", bufs=1, space="PSUM") as ps1,
        ):
            table1 = dpool.tile([NPAD, W12], F32)
            table2 = dpool.tile([NPAD, W12], F32)
            table3 = dpool.tile([NPAD, W3], F32)
            myrows2 = dpool.tile([CORE_ROWS, W12], F32)
            myrows3 = dpool.tile([CORE_ROWS, W3], F32)
            ar_in = dpool.tile([G, 64], F32)
            ar_out = dpool.tile([G, 64], F32)

            # ---- persistent SBUF
            idx_sb = pp.tile([128, Dsum], I32)
            nc.sync.dma_start(out=idx_sb[:], in_=idx_in[:])
            mask_sb = pp.tile([128, Dsum], F32)
            nc.sync.dma_start(out=mask_sb[:], in_=mask_in[:])
            cinv_sb = pp.tile([128, KBLK], F32)
            nc.sync.dma_start(out=cinv_sb[:], in_=cinv_in[:])
            ve_sb = pp.tile([128, 54], F32)
            nc.sync.dma_start(out=ve_sb[:], in_=ve9_in[:])
            t9 = pp.tile([128, 9 * Dsum], F32)
            b1_sb = pp.tile([128, 256], F32)
            nc.sync.dma_start(out=b1_sb[:], in_=b1_in[:])
            b2_sb = pp.tile([128, 256], F32)
            nc.sync.dma_start(out=b2_sb[:], in_=b2_in[:])
            b3_sb = pp.tile([128, 64], F32)
            nc.sync.dma_start(out=b3_sb[:], in_=b3_in[:])
            ident_sb = pp.tile([128, 128], F32)
            nc.sync.dma_start(out=ident_sb[:], in_=ident_in[:])
            rhs1_sb = pp.tile([7, W12], F32)
            nc.sync.dma_start(out=rhs1_sb[:], in_=rhs1_in[:])
            rhs2a_sb = pp.tile([128, W12], F32)
            nc.sync.dma_start(out=rhs2a_sb[:], in_=rhs2_in[0:128, :])
            rhs2b_sb = pp.tile([128, W12], F32)
            nc.sync.dma_start(out=rhs2b_sb[:], in_=rhs2_in[128:256, :])
            rhs3a_sb = pp.tile([128, W3], F32)
            nc.sync.dma_start(out=rhs3a_sb[:], in_=rhs3_in[0:128, :])
            rhs3b_sb = pp.tile([128, W3], F32)
            nc.sync.dma_start(out=rhs3b_sb[:], in_=rhs3_in[128:256, :])
            ginv_sb = pp.tile([G, 1], F32)
            nc.sync.dma_start(out=ginv_sb[:], in_=ginv_in[:])
            p1w_sb = pp.tile([64, 64], F32)
            nc.sync.dma_start(out=p1w_sb[:], in_=p1w_in[:])
            p2w_sb = pp.tile([64, 32], F32)
            nc.sync.dma_start(out=p2w_sb[:], in_=p2w_in[:])
            p3w_sb = pp.tile([32, 1], F32)
            nc.sync.dma_start(out=p3w_sb[:], in_=p3w_in[:])
            p1b_sb = pp.tile([G, 64], F32)
            nc.sync.dma_start(out=p1b_sb[:], in_=p1b_in[:])
            p2b_sb = pp.tile([G, 32], F32)
            nc.sync.dma_start(out=p2b_sb[:], in_=p2b_in[:])
            p3b_sb = pp.tile([G, 1], F32)
            nc.sync.dma_start(out=p3b_sb[:], in_=p3b_in[:])

            # ---- t9 pass: t9[p, l9, j] per block; l9 = (L1 h0..3, L2 h0..3, L3)
            for k in range(KBLK):
                dk = int(D[k])
                ea_k = wp.tile([128, dk * 6], F32, tag="ea")
                nc.sync.dma_start(out=ea_k[:], in_=ea_in[:, off[k] * 6: off[k + 1] * 6])
                tk = t9[:].rearrange("p (n j) -> p n j", n=9)[:, :, off[k]: off[k + 1]]
                tmp = wp.tile([128, 9 * dk], F32, tag="ttmp")
                tmpv = tmp[:].rearrange("p (n j) -> p n j", n=9)
                eav = ea_k[:].rearrange("p (j f) -> p f j", f=6)
                for f in range(6):
                    eab = eav[:, f, :].to_broadcast([128, dk, 9]).rearrange("p j n -> p n j")
                    veb = ve_sb[:, f * 9: (f + 1) * 9].to_broadcast([128, 9, dk])
                    if f == 0:
                        nc.vector.tensor_mul(tk, eab, veb)
                    else:
                        nc.vector.tensor_mul(tmpv, eab, veb)
                        nc.vector.tensor_add(tk, tk, tmpv)
                # self-loop slot 0 = (sum over slots) * cnt_inv
                tsum = wp.tile([128, 9], F32, tag="tsum")
                nc.vector.reduce_sum(tsum[:], tk, axis=AX)
                nc.vector.tensor_scalar(
                    out=tk[:, :, 0:1].rearrange("p n j -> p (n j)"),
                    in0=tsum[:], scalar1=cinv_sb[:, k: k + 1], scalar2=None,
                    op0=mybir.AluOpType.mult)
                # bake pad mask (broadcast over the 9 lanes)
                mb = mask_sb[:, off[k]: off[k + 1]].to_broadcast([128, dk, 9]).rearrange("p j n -> p n j")
                nc.vector.tensor_add(tk, tk, mb)

            # ---- per layer
            def projection1():
                CH = min(2048, NPAD)
                for ci in range(NPAD // CH):
                    xt_c = jp.tile([7, CH], F32, tag="xtc")
                    nc.sync.dma_start(out=xt_c[:], in_=xt_in[:, ci * CH: (ci + 1) * CH])
                    for m in range(CH // 128):
                        ps = psp.tile([128, W12], F32, tag="proj", space="PSUM")
                        nc.tensor.matmul(ps[:], lhsT=xt_c[:, m * 128: (m + 1) * 128],
                                         rhs=rhs1_sb[:], start=True, stop=True)
                        ot = jp.tile([128, W12], F32, tag="projo")
                        nc.vector.tensor_copy(ot[:], ps[:])
                        r0 = ci * CH + m * 128
                        nc.sync.dma_start(out=table1[r0: r0 + 128, :], in_=ot[:])

            def agg_layer(layer):
                wl, hl, hc = (W12, 4, 256) if layer < 3 else (W3, 1, 64)
                tbl = (table1, table2, table3)[layer - 1]
                l9o = 0 if layer == 1 else (4 if layer == 2 else 8)
                for k in range(KBLK):
                    dk = int(D[k])
                    hs = gp.tile([128, dk * wl], F32, tag="hs")
                    for j in range(dk):
                        nc.gpsimd.indirect_dma_start(
                            out=hs[:, j * wl: (j + 1) * wl], out_offset=None,
                            in_=tbl[:],
                            in_offset=bass.IndirectOffsetOnAxis(
                                ap=idx_sb[:, off[k] + j: off[k] + j + 1], axis=0))
                    hsv = hs[:].rearrange("p (j w) -> p j w", w=wl)
                    # alpha = t + s[src] + d[own]
                    a = wp.tile([128, hl * dk], F32, tag="alpha")
                    av = a[:].rearrange("p (h j) -> p h j", h=hl)
                    tk = t9[:].rearrange("p (n j) -> p n j", n=9)[:, l9o: l9o + hl,
                                                                 off[k]: off[k + 1]]
                    sv = hsv[:, :, hc: hc + hl].rearrange("p j h -> p h j")
                    nc.vector.tensor_add(av, tk, sv)
                    for h in range(hl):
                        nc.vector.tensor_scalar_add(
                            av[:, h, :], av[:, h, :],
                            hsv[:, 0, hc + hl + h: hc + hl + h + 1])
                    lr = wp.tile([128, hl * dk], F32, tag="lr")
                    nc.vector.tensor_scalar_mul(lr[:], a[:], 0.2)
                    nc.vector.tensor_max(a[:], a[:], lr[:])
                    mx = wp.tile([128, hl], F32, tag="mx")
                    nc.vector.reduce_max(mx[:], av, axis=AX)
                    negm = wp.tile([128, hl], F32, tag="negm")
                    nc.vector.tensor_scalar_mul(negm[:], mx[:], -1.0)
                    e = wp.tile([128, hl * dk], F32, tag="e")
                    ev = e[:].rearrange("p (h j) -> p h j", h=hl)
                    for h in range(hl):
                        nc.scalar.activation(e[:, h * dk: (h + 1) * dk],
                                             a[:, h * dk: (h + 1) * dk],
                                             AF.Exp, bias=negm[:, h: h + 1])
                    ssum = wp.tile([128, hl], F32, tag="ssum")
                    nc.vector.reduce_sum(ssum[:], ev, axis=AX)
                    nc.vector.tensor_scalar_add(ssum[:], ssum[:], 1e-16)
                    rinv = wp.tile([128, hl], F32, tag="rinv")
                    nc.vector.reciprocal(rinv[:], ssum[:])
                    # weighted sum over slots
                    prod_in = hsv[:, :, 0:hc]
                    eb = ev.rearrange("p h j -> p j h").to_broadcast([128, dk, hl, 64]) \
                        if hl > 1 else ev.rearrange("p h j -> p j h").to_broadcast([128, dk, 1, 64])
                    pv = prod_in.rearrange("p j (h c) -> p j h c", h=hl)
                    nc.vector.tensor_mul(pv, pv, eb)
                    n = dk
                    while n > 1:
                        if n % 2 == 1:
                            nc.vector.tensor_add(hsv[:, 0, 0:hc], hsv[:, 0, 0:hc],
                                                 hsv[:, n - 1, 0:hc])
                            n -= 1
                        half = n // 2
                        nc.vector.tensor_add(hsv[:, 0:half, 0:hc],
                                             hsv[:, 0:half, 0:hc],
                                             hsv[:, half: n, 0:hc])
                        n = half
                    y = wp.tile([128, hc], F32, tag="y")
                    rb = rinv[:].to_broadcast([128, hl, 64])
                    nc.vector.tensor_mul(y[:].rearrange("p (h c) -> p h c", c=64),
                                         hsv[:, 0, 0:hc].rearrange("p (h c) -> p h c", c=64),
                                         rb)
                    # bias + elu
                    bt = (b1_sb, b2_sb, b3_sb)[layer - 1]
                    nc.vector.tensor_add(y[:], y[:], bt[:, 0:hc])
                    u = wp.tile([128, hc], F32, tag="u")
                    nc.vector.tensor_scalar_min(u[:], y[:], 0.0)
                    eu = wp.tile([128, hc], F32, tag="eu")
                    nc.scalar.activation(eu[:], u[:], AF.Exp)
                    nc.vector.tensor_scalar_max(y[:], y[:], 0.0)
                    nc.vector.tensor_add(y[:], y[:], eu[:])
                    nc.vector.tensor_scalar_add(y[:], y[:], -1.0)
                    if layer < 3:
                        # project next-layer table rows for own nodes:
                        # y^T via PE transpose, then (y @ rhs_next) [128, wnext]
                        ra, rb = (rhs2a_sb, rhs2b_sb) if layer == 1 else (rhs3a_sb, rhs3b_sb)
                        wnext = W12 if layer == 1 else W3
                        myrows = myrows2 if layer == 1 else myrows3
                        pt = psp.tile([128, wnext], F32, tag="proj", space="PSUM")
                        for half in range(2):
                            tp = psp.tile([128, 128], F32, tag="tp", space="PSUM")
                            nc.tensor.transpose(tp[:], y[:, half * 128: (half + 1) * 128],
                                                ident_sb[:])
                            to = wp.tile([128, 128], F32, tag="tpo")
                            nc.vector.tensor_copy(to[:], tp[:])
                            nc.tensor.matmul(pt[:], lhsT=to[:], rhs=(ra if half == 0 else rb)[:],
                                             start=(half == 0), stop=(half == 1))
                        ot = jp.tile([128, wnext], F32, tag="projo")
                        nc.vector.tensor_copy(ot[:], pt[:])
                        nc.sync.dma_start(out=myrows[k * 128: (k + 1) * 128, :], in_=ot[:])
                        if (k + 1) % KCH == 0:
                            ch = k // KCH
                            tbln = table2 if layer == 1 else table3
                            nc.gpsimd.collective_compute(
                                "AllGather", mybir.AluOpType.bypass,
                                replica_groups=[list(range(NCORES))],
                                ins=[myrows[ch * KCH * 128: (ch + 1) * KCH * 128, :].opt()],
                                outs=[tbln[ch * NCORES * KCH * 128:
                                           (ch + 1) * NCORES * KCH * 128, :].opt()])
                    else:
                        oh = wp.tile([128, G], F32, tag="oh")
                        nc.sync.dma_start(out=oh[:], in_=oneh_in[k * 128: (k + 1) * 128, :])
                        nc.tensor.matmul(racc_ps[:], lhsT=oh[:], rhs=y[:],
                                         start=(k == 0), stop=(k == KBLK - 1))

            # ===== layers (projection of next table fused into aggregation)
            projection1()
            agg_layer(1)
            agg_layer(2)
            racc_ps = ps1.tile([G, 64], F32, tag="racc", space="PSUM")
            agg_layer(3)
            racc = wp.tile([G, 64], F32, tag="racc_sb")
            nc.vector.tensor_copy(racc[:], racc_ps[:])
            nc.sync.dma_start(out=ar_in[:], in_=racc[:])
            nc.gpsimd.collective_compute(
                "AllReduce", mybir.AluOpType.add,
                replica_groups=[list(range(NCORES))],
                ins=[ar_in.opt()], outs=[ar_out.opt()])
            ro = wp.tile([G, 64], F32, tag="ro")
            nc.sync.dma_start(out=ro[:], in_=ar_out[:])
            nc.vector.tensor_scalar(out=ro[:], in0=ro[:], scalar1=ginv_sb[:, 0:1],
                                    scalar2=None, op0=mybir.AluOpType.mult)
            nc.sync.dma_start(out=ro_out[:], in_=ro[:])
            # ---- MLP head (replicated on every core)
            def ptranspose(src_ap, cols, parts):
                tp = psp.tile([128, 128], F32, tag="tp", space="PSUM")
                nc.tensor.transpose(tp[:cols, :parts], src_ap, ident_sb[:parts, :parts])
                to = wp.tile([128, 128], F32, tag="tpo")
                nc.vector.tensor_copy(to[:cols, :parts], tp[:cols, :parts])
                return to

            rt = ptranspose(ro[:], 64, G)
            z1ps = psp.tile([G, 64], F32, tag="mlp", space="PSUM")
            nc.tensor.matmul(z1ps[:], lhsT=rt[:64, :G], rhs=p1w_sb[:], start=True, stop=True)
            z1 = wp.tile([G, 64], F32, tag="z1")
            nc.vector.tensor_add(z1[:], z1ps[:], p1b_sb[:])
            nc.scalar.activation(z1[:], z1[:], AF.Relu)
            z1t = ptranspose(z1[:], 64, G)
            z2ps = psp.tile([G, 32], F32, tag="mlp", space="PSUM")
            nc.tensor.matmul(z2ps[:], lhsT=z1t[:64, :G], rhs=p2w_sb[:], start=True, stop=True)
            z2 = wp.tile([G, 32], F32, tag="z2")
            nc.vector.tensor_add(z2[:], z2ps[:], p2b_sb[:])
            nc.scalar.activation(z2[:], z2[:], AF.Relu)
            z2t = ptranspose(z2[:], 32, G)
            z3ps = psp.tile([G, 1], F32, tag="mlp", space="PSUM")
            nc.tensor.matmul(z3ps[:], lhsT=z2t[:32, :G], rhs=p3w_sb[:], start=True, stop=True)
            z3 = wp.tile([G, 1], F32, tag="z3")
            nc.vector.tensor_add(z3[:], z3ps[:], p3b_sb[:])
            nc.sync.dma_start(out=og_out[:], in_=z3[:])

    nc.compile()
    return nc


# ---------------------------------------------------------------- entry point

def kernel(**inputs):
    ip = {k: np.asarray(v) for k, v in inputs.items()}
    L = build_layout(ip["edge_index"], ip["batch"])
    rhs1, rhs2, rhs3, ve9 = derived_weights(ip)
    ea = fill_ea(L, ip["edge_attr"])

    x_rows = np.zeros((NPAD, 7), np.float32)
    valid = L["node_of_row"] >= 0
    x_rows[valid] = np.asarray(ip["x"], np.float32)[L["node_of_row"][valid]]
    xt = np.ascontiguousarray(x_rows.T)

    key = "prog"
    if key not in _CACHE:
        _CACHE[key] = build_program(L)
    nc = _CACHE[key]

    rep = lambda v, w: np.broadcast_to(np.asarray(v, np.float32)[None, :], (128, w)).copy()
    repg = lambda v, w: np.broadcast_to(np.asarray(v, np.float32)[None, :], (G, w)).copy()
    shared = {
        "xt": xt, "rhs1": rhs1, "rhs2": rhs2, "rhs3": rhs3,
        "ve9": np.broadcast_to(ve9.reshape(1, 54), (128, 54)).copy(),
        "ginv": L["gcnt_inv"].reshape(G, 1),
        "b1t": rep(ip["b1"], 256), "b2t": rep(ip["b2"], 256),
        "b3t": rep(ip["b3"], 64),
        "ident": np.eye(128, dtype=np.float32),
        "p1w": np.asarray(ip["p1w"], np.float32),
        "p2w": np.asarray(ip["p2w"], np.float32),
        "p3w": np.asarray(ip["p3w"], np.float32),
        "p1bt": repg(ip["p1b"], 64), "p2bt": repg(ip["p2b"], 32),
        "p3bt": repg(ip["p3b"], 1),
    }
    in_maps = []
    for c in range(NCORES):
        m = dict(shared)
        m["idx"] = L["idx"][c]
        m["mask"] = L["mask"][c]
        m["ea"] = ea[c]
        m["cinv"] = L["cnt_inv"][c]
        m["oneh"] = L["onehot"][c].reshape(KBLK * 128, G)
        in_maps.append(m)

    trace = bool(os.environ.get("GAT_TRACE"))
    res = run_bass_kernel_spmd(nc, in_maps, list(range(NCORES)), trace=trace)
    global last_exec_time_ns
    last_exec_time_ns = res.exec_time_ns
    out = res.results[0]["out_g"][:, 0]
    readout = res.results[0]["readout"]
    return out, readout


if __name__ == "__main__":
    dat = np.load("/root/problem/inputs.npz")
    inputs = {k: dat[k] for k in dat.files}
    exp = np.load("/root/problem/expected.npz")
    out, readout = kernel(**inputs)
    ro = np.abs(readout - exp["readout"]).max() / np.abs(exp["readout"]).max()
    oo = np.abs(out - exp["out"]).max() / np.abs(exp["out"]).max()
    print("readout rel err:", ro)
    print("out rel err:", oo)
